# revision 40
# baseline (speedup 1.0000x reference)
"""DCT block extractor kernel for 8 TRN2 NeuronCores (pure data parallel).

Math: for each 8x8 block of each [512,512] image, the 2D-DFT bin (u,v) is
  X[u,v] = sum_{r,s} x[r,s] * exp(-2*pi*i*(u*r + v*s)/8)
We need |X| at 6 (u,v) bands, averaged over all 64x64 blocks.

The kernel is HBM-stream-bound: each core must read its 25.2 MB fp32 shard
once (~62 us at the measured ~400 GB/s per-core DMA rate).  Everything else
is sized to stay strictly below that rate so the wall time is
(stream + small startup + short drain):

- Input DMAs (gpsimd software-DGE) cast fp32 -> fp8e4 in flight.  All tile
  DMAs are issued up-front; the whole 6.3 MB fp8 shard stays resident in
  SBUF, so the HBM stream never stalls on buffer reuse.
- The weights hold only values {0, +-1, +-sqrt2/2}; 0/+-1 are exact in fp8
  and the +-sqrt2/2 entries are rounded up/down (0.75/0.6875) in a
  compensated pattern per band so the first-order bias sum(w * dw) cancels.
  Measured end-to-end max rel err ~1.8e-3 (vs 2e-2 budget).
- Matmuls run in fp8 DoubleRow perf mode: each matmul contracts TWO
  in-block column offsets s (k-tiles) per pass, so a 64-row chunk needs
  only 4 matmuls (half the fp16 formulation's PE time; the k=64 x 2-ktile
  shape also fills the full 128-row PE array).  This keeps the PE below
  the DMA rate even when the HAM clock gate holds it at 1.2 GHz, so no
  backlog builds up and the post-stream drain is one short chunk chain.
  The raw DMA tile layout ([rows, img, (gj,s)]) is consumed directly: the
  DoubleRow pair (s, s+1) is 2 adjacent bytes read at stride 8 - no
  deinterleave pass at all.  The last chunk's matmuls and magnitude chain
  are split into two gj-halves so compute overlaps the final half-DMA.
- Magnitude: the matmul writes Re at PSUM partitions 0:48 (band*8+gi) and
  Im at 64:112 (rest zero); ACT squares each half to a base-0 tile, the DVE
  adds re^2+im^2, and two chunks share one Sqrt / reduce pass by packing
  the chunk pair along the free dim ([64, 2, 512]).
Final tiny mean/reshape is done on host from a [64, 3, 2, 8] per-core
result.
"""

import os
import sys
from contextlib import nullcontext

import numpy as np

for _p in ("/opt/trn_rl_repo",):
    if os.path.isdir(_p) and _p not in sys.path:
        sys.path.insert(0, _p)

import ml_dtypes  # noqa: E402

import concourse.bass as bass  # noqa: E402
import concourse.tile as tile  # noqa: E402
from concourse import bacc, mybir  # noqa: E402
from concourse.bass_utils import run_bass_kernel_spmd  # noqa: E402

# Problem shape (hardcoded per contract)
B, C, H, W = 64, 3, 512, 512
N_CORES = 8
BL = B // N_CORES   # 8 batch rows per core
NIMG = BL * C       # 24 images per core (flattened (b, c))
IPB = 8             # images per device-batch
NBATCH = NIMG // IPB  # 3 device-batches
NCHUNK = 8          # 64-row chunks per image
GJ = 64             # block-columns
NFREE = IPB * GJ    # 512 matmul free size
NBANDS = 6
MOUT = 128          # PSUM partitions: Re at 0:48, Im at 64:112, rest zero
# Input tiles as (first global chunk, n chunks): 2-chunk tiles for the bulk,
# 1-chunk tiles at the end so the pipeline drain after the last HBM byte is
# one short chunk chain instead of a 2-chunk one.  The last chunk's DMA is
# further split into two half-width (gj) pieces so its matmuls start before
# the final bytes land.
TILES = [(2 * t, 2) for t in range(10)] + [(20 + k, 1) for k in range(4)]
DRAIN_C0 = 22   # chunks >= this get the latency-optimized drain treatment

FREQ_BANDS = np.array([[0, 1], [1, 0], [1, 1], [2, 2], [3, 3], [4, 4]]) % 8

BENCH = False          # set True (e.g. from test.py) to profile
BENCH_KWARGS = {}
LAST_EXEC_NS = None
LAST_RESULTS = None

_CACHED_NC = None


def _comp_quant(vals: np.ndarray) -> np.ndarray:
    """Quantize cos/sin weights to fp8e4m3 with compensated rounding.

    All values are in {0, +-1, +-sqrt2/2}; only +-sqrt2/2 is inexact
    (0.6875 below, 0.75 above).  Choose up/down per entry (greedy) to drive
    the first-order bias sum_i v_i*(q_i - v_i) to ~0, which removes the
    systematic |X| shrinkage that naive nearest-rounding (-2.77% on every
    sqrt2/2 entry) would cause.
    """
    flat = vals.astype(np.float64).ravel().copy()
    flat[np.abs(flat) < 1e-8] = 0.0
    q8 = lambda a: np.asarray(a, np.float32).astype(ml_dtypes.float8_e4m3).astype(np.float64)
    qdn = np.empty_like(flat)
    qup = np.empty_like(flat)
    for i, v in enumerate(flat):
        n = float(q8(v))
        if n == v:
            qdn[i] = qup[i] = v
            continue
        lo, hi = (0.6875, 0.75)
        m = abs(v)
        assert abs(m - np.sqrt(2) / 2) < 1e-12, v
        s = 1.0 if v > 0 else -1.0
        qdn[i], qup[i] = s * lo, s * hi
    q = np.where(np.abs(qdn - flat) <= np.abs(qup - flat), qdn, qup)

    def bias(qq):
        return float(np.sum(flat * (qq - flat)))

    for _ in range(500):
        b = bias(q)
        best = None
        for i in range(len(flat)):
            alt = qup[i] if q[i] == qdn[i] else qdn[i]
            if alt == q[i]:
                continue
            nb = b + flat[i] * (alt - q[i])
            if abs(nb) < abs(b) - 1e-15 and (best is None or abs(nb) < best[0]):
                best = (abs(nb), i, alt)
        if best is None:
            break
        q[best[1]] = best[2]
    return q.reshape(vals.shape).astype(np.float32)


def _weights() -> np.ndarray:
    """W in [128, 4, 2, 128] fp8e4: [k, s_pair, t, m] with s = 2*s_pair + t.

    m: Re at band*8+gi (0:48), Im at 64+band*8+gi (64:112); k = gi*8+r
    block-diagonal over the 8 row-groups.  Rows 64:128 duplicate rows 0:64
    so lhsT can be sliced at base partition 0 or 64 to match the rhs chunk's
    base partition."""
    r = np.arange(8)
    w = np.zeros((64, 8, MOUT), dtype=np.float32)  # [k, s, m]
    for b, (u, v) in enumerate(FREQ_BANDS):
        th = 2.0 * np.pi * (u * r[:, None] + v * r[None, :]) / 8.0
        cs = _comp_quant(np.concatenate([np.cos(th), np.sin(th)], axis=1))
        cw, sw = cs[:, :8], cs[:, 8:]
        for gi in range(8):
            w[gi * 8 : gi * 8 + 8, :, b * 8 + gi] = cw
            w[gi * 8 : gi * 8 + 8, :, 64 + b * 8 + gi] = sw
    w = w.reshape(64, 4, 2, MOUT)
    w = np.concatenate([w, w], axis=0)  # duplicate for base partition 64
    return np.ascontiguousarray(w.astype(ml_dtypes.float8_e4m3))


def _build(num_devices: int = N_CORES):
    nc = bacc.Bacc(
        "TRN2", target_bir_lowering=False, debug=False, num_devices=num_devices
    )
    f32 = mybir.dt.float32
    f16 = mybir.dt.float16
    f8 = mybir.dt.float8e4

    x_d = nc.dram_tensor("x", [NIMG, H, W], f32, kind="ExternalInput")
    w_d = nc.dram_tensor("w", [128, 4, 2, MOUT], f8, kind="ExternalInput")
    out_d = nc.dram_tensor("out", [64, NBATCH, 2, IPB], f32, kind="ExternalOutput")

    with tile.TileContext(nc) as tc:
        with (
            tc.tile_pool(name="consts", bufs=1) as consts,
            tc.tile_pool(name="inp", bufs=len(TILES)) as inp,
            tc.tile_pool(name="psum", bufs=1, space="PSUM") as psum_pool,
            tc.tile_pool(name="sqp", bufs=4) as sqp,
            tc.tile_pool(name="ssp", bufs=3) as ssp,
            tc.tile_pool(name="magp", bufs=3) as magp,
            tc.tile_pool(name="redp", bufs=3) as redp,
        ):
            w_sb = consts.tile([128, 4, 2, MOUT], f8)
            nc.sync.dma_start(out=w_sb, in_=w_d[:])

            # All input DMAs up-front: the tiles cover the whole per-core
            # input, so the 16 DMA queues stream HBM back-to-back.  One DMA
            # per tile (64/128 consecutive image rows -> partitions); the
            # software-DGE DMA casts fp32 -> fp8e4 in flight.  The very last
            # tile is two half-width DMAs so the final matmul group only
            # waits on the last half of the stream.
            in_tiles = []
            for ti, (c0, nch) in enumerate(TILES):
                bt, cb = divmod(c0, NCHUNK)
                it = inp.tile([64 * nch, IPB, W], f8, name="it")
                rows = x_d[
                    bt * IPB : (bt + 1) * IPB,
                    cb * 64 : (cb + nch) * 64,
                    :,
                ]
                if ti == len(TILES) - 1:
                    for h in range(2):
                        nc.gpsimd.dma_start(
                            out=it[:, :, 256 * h : 256 * (h + 1)],
                            in_=rows[:, :, 256 * h : 256 * (h + 1)].transpose(
                                [1, 0, 2]
                            ),
                        )
                else:
                    nc.gpsimd.dma_start(out=it, in_=rows.transpose([1, 0, 2]))
                in_tiles.append(it)

            # out_acc columns: (batch, chunk-parity, img-in-batch)
            out_acc = consts.tile([64, NBATCH, 2, IPB], f32)

            ss_tiles = {}  # pair index -> ss tile [64, 2(parity), 8, 64]
            for jt, (c0, nch) in enumerate(TILES):
                it = in_tiles[jt]
                for k in range(nch):
                    c = c0 + k
                    bt = c // NCHUNK
                    base = 64 * k
                    drain = c >= DRAIN_C0
                    last = c == NIMG // IPB * NCHUNK - 1
                    ps = psum_pool.tile([MOUT, NFREE], f32, tag=f"ps{c % 8}", name="ps")
                    # rhs pair (s, s+1) = 2 adjacent fp8 bytes; stream
                    # stride 8 bytes -> full-rate xbus reads, no deint.
                    rhs_v = it[base : base + 64].rearrange(
                        "p i (g sp t) -> p sp t i g", sp=4, t=2
                    )
                    ps_v = ps.rearrange("m (i g) -> m i g", g=GJ)
                    # The last chunk runs its matmuls in two gj-half groups,
                    # so the first group only needs the first half-DMA.
                    halves = (
                        [(0, 32), (32, 64)] if last else [(0, GJ)]
                    )
                    for g0, g1 in halves:
                        for sp in range(4):
                            nc.tensor.matmul(
                                ps_v[:, :, g0:g1],
                                w_sb[base : base + 64, sp],
                                rhs_v[:, sp, :, :, g0:g1],
                                start=(sp == 0),
                                stop=(sp == 3),
                                perf_mode=mybir.MatmulPerfMode.DoubleRow,
                            )
                    # |X| = sqrt(re^2 + im^2).  DVE TensorTensor requires
                    # equal SBUF base partitions (and at most one PSUM
                    # input), so the Re/Im squares go through ACT into two
                    # base-0 tiles and the DVE adds them.  Rows 48:64 are
                    # zero lhsT columns -> always initialized.
                    # Priority-bias the near-drain chunks' magnitude
                    # chains: without this the scheduler orders the
                    # (20,21) pair sqrt/reduce AFTER chunk 22/23's squares
                    # on the ACT/DVE queues, and the serial batch-2
                    # accumulation chain stretches the post-stream tail by
                    # ~2us (measured).
                    hp = (
                        tc.high_priority(offset=30)
                        if DRAIN_C0 - 2 <= c < 23
                        else nullcontext()
                    )
                    hp.__enter__()
                    sq_re = sqp.tile([64, NFREE], f16)
                    sq_im = sqp.tile([64, NFREE], f16)
                    nc.scalar.square(sq_re, ps[0:64])
                    nc.scalar.square(sq_im, ps[64:128])
                    # Chunk pairs share one Sqrt/reduce pass (packed along
                    # the ss free dim); the drain chunks are unpaired so the
                    # final dependency chain is short.
                    if drain:
                        ss1 = ssp.tile([64, IPB, GJ], f16, name="ss1")
                        nc.vector.tensor_add(
                            ss1.rearrange("p i g -> p (i g)"), sq_re, sq_im
                        )
                        mag1 = magp.tile([64, IPB, GJ], f16)
                        nc.scalar.sqrt(mag1, ss1)
                        red1 = redp.tile([64, IPB], f32)
                        nc.vector.reduce_sum(
                            out=red1, in_=mag1, axis=mybir.AxisListType.X
                        )
                        ocol = out_acc[:, bt, c % 2]
                        nc.vector.tensor_add(ocol, ocol, red1)
                    else:
                        pair = c // 2
                        if pair not in ss_tiles:
                            ss_tiles[pair] = ssp.tile(
                                [64, 2, IPB, GJ], f16, name="ss"
                            )
                        ss = ss_tiles[pair]
                        nc.vector.tensor_add(
                            ss[:, c % 2].rearrange("p i g -> p (i g)"),
                            sq_re,
                            sq_im,
                        )
                        if c % 2 == 1:
                            mag = magp.tile([64, 2, IPB, GJ], f16)
                            nc.scalar.sqrt(mag, ss)
                            ocol = out_acc[:, bt]
                            if c % NCHUNK == 1:
                                nc.vector.reduce_sum(
                                    out=ocol, in_=mag, axis=mybir.AxisListType.X
                                )
                            else:
                                red = redp.tile([64, 2, IPB], f32)
                                nc.vector.reduce_sum(
                                    out=red, in_=mag, axis=mybir.AxisListType.X
                                )
                                nc.vector.tensor_add(ocol, ocol, red)
                    hp.__exit__(None, None, None)
                    # Per-batch output DMA: batches 0/1 fly mid-stream; only
                    # batch 2's small DMA trails the final accumulation.
                    if c % NCHUNK == NCHUNK - 1:
                        nc.sync.dma_start(
                            out=out_d[:, bt], in_=out_acc[:, bt]
                        )

    nc.compile()
    return nc


def kernel(x: np.ndarray) -> np.ndarray:
    global _CACHED_NC, LAST_EXEC_NS, LAST_RESULTS
    x = np.ascontiguousarray(np.asarray(x, dtype=np.float32))
    assert x.shape == (B, C, H, W), x.shape

    if _CACHED_NC is None:
        _CACHED_NC = _build()
    nc = _CACHED_NC

    w = _weights()
    in_maps = [
        {"x": x[i * BL : (i + 1) * BL].reshape(NIMG, H, W), "w": w}
        for i in range(N_CORES)
    ]
    kwargs = dict(BENCH_KWARGS)
    if BENCH:
        kwargs.setdefault("trace", True)
    res = run_bass_kernel_spmd(nc, in_maps, core_ids=list(range(N_CORES)), **kwargs)
    LAST_EXEC_NS = res.exec_time_ns
    LAST_RESULTS = res

    outs = []
    for i in range(N_CORES):
        # [64, NBATCH, 2, IPB]: row = band*8+gi (rows 48:64 zero),
        # cols = (batch, chunk-parity, img-in-batch)
        o = np.asarray(res.results[i]["out"], dtype=np.float64)
        o = o[0:48].reshape(NBANDS, 8, NBATCH, 2, IPB)
        tot = o.sum(axis=(1, 3)).reshape(NBANDS, NIMG) / 4096.0  # [band, img]
        outs.append(tot.T.reshape(BL, C * NBANDS))  # img = b_l*C + ch
    return np.concatenate(outs, axis=0).astype(np.float32)


# revision 42
# speedup vs baseline: 1.0778x; 1.0778x over previous
"""DCT block extractor kernel for 8 TRN2 NeuronCores (pure data parallel).

Math: for each 8x8 block of each [512,512] image, the 2D-DFT bin (u,v) is
  X[u,v] = sum_{r,s} x[r,s] * exp(-2*pi*i*(u*r + v*s)/8)
We need |X| at 6 (u,v) bands, averaged over all 64x64 blocks.

The kernel is HBM-stream-bound: each core must read its 25.2 MB fp32 shard
once (~62 us at the measured ~400 GB/s per-core DMA rate).  Everything else
is sized to stay strictly below that rate so the wall time is
(stream + small startup + short drain):

- Input DMAs (gpsimd software-DGE) cast fp32 -> fp8e4 in flight.  All tile
  DMAs are issued up-front; the whole 6.3 MB fp8 shard stays resident in
  SBUF, so the HBM stream never stalls on buffer reuse.
- The weights hold only values {0, +-1, +-sqrt2/2}; 0/+-1 are exact in fp8
  and the +-sqrt2/2 entries are rounded up/down (0.75/0.6875) in a
  compensated pattern per band so the first-order bias sum(w * dw) cancels.
  Measured end-to-end max rel err ~1.8e-3 (vs 2e-2 budget).
- Matmuls run in fp8 DoubleRow perf mode: each matmul contracts TWO
  in-block column offsets s (k-tiles) per pass, so a 64-row chunk needs
  only 4 matmuls (half the fp16 formulation's PE time; the k=64 x 2-ktile
  shape also fills the full 128-row PE array).  This keeps the PE below
  the DMA rate even when the HAM clock gate holds it at 1.2 GHz, so no
  backlog builds up and the post-stream drain is one short chunk chain.
  The raw DMA tile layout ([rows, img, (gj,s)]) is consumed directly: the
  DoubleRow pair (s, s+1) is 2 adjacent bytes read at stride 8 - no
  deinterleave pass at all.  The last chunk's matmuls and magnitude chain
  are split into two gj-halves so compute overlaps the final half-DMA.
- Magnitude: the matmul writes Re at PSUM partitions 0:48 (band*8+gi) and
  Im at 64:112 (rest zero); ACT squares each half to a base-0 tile, the DVE
  adds re^2+im^2, and two chunks share one Sqrt / reduce pass by packing
  the chunk pair along the free dim ([64, 2, 512]).
Final tiny mean/reshape is done on host from a [64, 3, 2, 8] per-core
result.
"""

import os
import sys

import numpy as np

for _p in ("/opt/trn_rl_repo",):
    if os.path.isdir(_p) and _p not in sys.path:
        sys.path.insert(0, _p)

import ml_dtypes  # noqa: E402

import concourse.bass as bass  # noqa: E402
import concourse.tile as tile  # noqa: E402
from concourse import bacc, mybir  # noqa: E402
from concourse.bass_utils import run_bass_kernel_spmd  # noqa: E402

# Problem shape (hardcoded per contract)
B, C, H, W = 64, 3, 512, 512
N_CORES = 8
BL = B // N_CORES   # 8 batch rows per core
NIMG = BL * C       # 24 images per core (flattened (b, c))
IPB = 8             # images per device-batch
NBATCH = NIMG // IPB  # 3 device-batches
NCHUNK = 8          # 64-row chunks per image
GJ = 64             # block-columns
NFREE = IPB * GJ    # 512 matmul free size
NBANDS = 6
MOUT = 128          # PSUM partitions: Re at 0:48, Im at 64:112, rest zero
# Input tiles as (first global chunk, n chunks): 2-chunk tiles for the bulk,
# 1-chunk tiles at the end so the pipeline drain after the last HBM byte is
# one short chunk chain instead of a 2-chunk one.  The last chunk's DMA is
# further split into two half-width (gj) pieces so its matmuls start before
# the final bytes land.
TILES = [(2 * t, 2) for t in range(10)] + [(20 + k, 1) for k in range(4)]
DRAIN_C0 = 20   # chunks >= this get the latency-optimized drain treatment

FREQ_BANDS = np.array([[0, 1], [1, 0], [1, 1], [2, 2], [3, 3], [4, 4]]) % 8

BENCH = False          # set True (e.g. from test.py) to profile
BENCH_KWARGS = {}
LAST_EXEC_NS = None
LAST_RESULTS = None

_CACHED_NC = None


def _comp_quant(vals: np.ndarray) -> np.ndarray:
    """Quantize cos/sin weights to fp8e4m3 with compensated rounding.

    All values are in {0, +-1, +-sqrt2/2}; only +-sqrt2/2 is inexact
    (0.6875 below, 0.75 above).  Choose up/down per entry (greedy) to drive
    the first-order bias sum_i v_i*(q_i - v_i) to ~0, which removes the
    systematic |X| shrinkage that naive nearest-rounding (-2.77% on every
    sqrt2/2 entry) would cause.
    """
    flat = vals.astype(np.float64).ravel().copy()
    flat[np.abs(flat) < 1e-8] = 0.0
    q8 = lambda a: np.asarray(a, np.float32).astype(ml_dtypes.float8_e4m3).astype(np.float64)
    qdn = np.empty_like(flat)
    qup = np.empty_like(flat)
    for i, v in enumerate(flat):
        n = float(q8(v))
        if n == v:
            qdn[i] = qup[i] = v
            continue
        lo, hi = (0.6875, 0.75)
        m = abs(v)
        assert abs(m - np.sqrt(2) / 2) < 1e-12, v
        s = 1.0 if v > 0 else -1.0
        qdn[i], qup[i] = s * lo, s * hi
    q = np.where(np.abs(qdn - flat) <= np.abs(qup - flat), qdn, qup)

    def bias(qq):
        return float(np.sum(flat * (qq - flat)))

    for _ in range(500):
        b = bias(q)
        best = None
        for i in range(len(flat)):
            alt = qup[i] if q[i] == qdn[i] else qdn[i]
            if alt == q[i]:
                continue
            nb = b + flat[i] * (alt - q[i])
            if abs(nb) < abs(b) - 1e-15 and (best is None or abs(nb) < best[0]):
                best = (abs(nb), i, alt)
        if best is None:
            break
        q[best[1]] = best[2]
    return q.reshape(vals.shape).astype(np.float32)


def _weights() -> np.ndarray:
    """W in [128, 4, 2, 128] fp8e4: [k, s_pair, t, m] with s = 2*s_pair + t.

    m: Re at band*8+gi (0:48), Im at 64+band*8+gi (64:112); k = gi*8+r
    block-diagonal over the 8 row-groups.  Rows 64:128 duplicate rows 0:64
    so lhsT can be sliced at base partition 0 or 64 to match the rhs chunk's
    base partition."""
    r = np.arange(8)
    w = np.zeros((64, 8, MOUT), dtype=np.float32)  # [k, s, m]
    for b, (u, v) in enumerate(FREQ_BANDS):
        th = 2.0 * np.pi * (u * r[:, None] + v * r[None, :]) / 8.0
        cs = _comp_quant(np.concatenate([np.cos(th), np.sin(th)], axis=1))
        cw, sw = cs[:, :8], cs[:, 8:]
        for gi in range(8):
            w[gi * 8 : gi * 8 + 8, :, b * 8 + gi] = cw
            w[gi * 8 : gi * 8 + 8, :, 64 + b * 8 + gi] = sw
    w = w.reshape(64, 4, 2, MOUT)
    w = np.concatenate([w, w], axis=0)  # duplicate for base partition 64
    return np.ascontiguousarray(w.astype(ml_dtypes.float8_e4m3))


def _build(num_devices: int = N_CORES):
    nc = bacc.Bacc(
        "TRN2", target_bir_lowering=False, debug=False, num_devices=num_devices
    )
    f32 = mybir.dt.float32
    f16 = mybir.dt.float16
    f8 = mybir.dt.float8e4

    x_d = nc.dram_tensor("x", [NIMG, H, W], f32, kind="ExternalInput")
    w_d = nc.dram_tensor("w", [128, 4, 2, MOUT], f8, kind="ExternalInput")
    out_d = nc.dram_tensor("out", [64, NBATCH, 2, IPB], f32, kind="ExternalOutput")

    with tile.TileContext(nc) as tc:
        with (
            tc.tile_pool(name="consts", bufs=1) as consts,
            tc.tile_pool(name="inp", bufs=len(TILES)) as inp,
            tc.tile_pool(name="psum", bufs=1, space="PSUM") as psum_pool,
            tc.tile_pool(name="sqp", bufs=4) as sqp,
            tc.tile_pool(name="ssp", bufs=3) as ssp,
            tc.tile_pool(name="magp", bufs=3) as magp,
            tc.tile_pool(name="redp", bufs=3) as redp,
        ):
            w_sb = consts.tile([128, 4, 2, MOUT], f8)
            nc.sync.dma_start(out=w_sb, in_=w_d[:])

            # All input DMAs up-front: the tiles cover the whole per-core
            # input, so the 16 DMA queues stream HBM back-to-back.  One DMA
            # per tile (64/128 consecutive image rows -> partitions); the
            # software-DGE DMA casts fp32 -> fp8e4 in flight.  The very last
            # tile is two half-width DMAs so the final matmul group only
            # waits on the last half of the stream.
            in_tiles = []
            for ti, (c0, nch) in enumerate(TILES):
                bt, cb = divmod(c0, NCHUNK)
                it = inp.tile([64 * nch, IPB, W], f8, name="it")
                rows = x_d[
                    bt * IPB : (bt + 1) * IPB,
                    cb * 64 : (cb + nch) * 64,
                    :,
                ]
                if ti == len(TILES) - 1:
                    for h in range(2):
                        nc.gpsimd.dma_start(
                            out=it[:, :, 256 * h : 256 * (h + 1)],
                            in_=rows[:, :, 256 * h : 256 * (h + 1)].transpose(
                                [1, 0, 2]
                            ),
                        )
                else:
                    nc.gpsimd.dma_start(out=it, in_=rows.transpose([1, 0, 2]))
                in_tiles.append(it)

            # out_acc columns: (batch, chunk-parity, img-in-batch)
            out_acc = consts.tile([64, NBATCH, 2, IPB], f32)

            ss_tiles = {}  # pair index -> ss tile [64, 2(parity), 8, 64]
            for jt, (c0, nch) in enumerate(TILES):
                it = in_tiles[jt]
                for k in range(nch):
                    c = c0 + k
                    bt = c // NCHUNK
                    base = 64 * k
                    drain = c >= DRAIN_C0
                    last = c == NIMG // IPB * NCHUNK - 1
                    ps = psum_pool.tile([MOUT, NFREE], f32, tag=f"ps{c % 8}", name="ps")
                    # rhs pair (s, s+1) = 2 adjacent fp8 bytes; stream
                    # stride 8 bytes -> full-rate xbus reads, no deint.
                    rhs_v = it[base : base + 64].rearrange(
                        "p i (g sp t) -> p sp t i g", sp=4, t=2
                    )
                    ps_v = ps.rearrange("m (i g) -> m i g", g=GJ)
                    # The last chunk runs its matmuls in two gj-half groups,
                    # so the first group only needs the first half-DMA.
                    halves = (
                        [(0, 32), (32, 64)] if last else [(0, GJ)]
                    )
                    for g0, g1 in halves:
                        for sp in range(4):
                            nc.tensor.matmul(
                                ps_v[:, :, g0:g1],
                                w_sb[base : base + 64, sp],
                                rhs_v[:, sp, :, :, g0:g1],
                                start=(sp == 0),
                                stop=(sp == 3),
                                perf_mode=mybir.MatmulPerfMode.DoubleRow,
                            )
                    # |X| = sqrt(re^2 + im^2).  DVE TensorTensor requires
                    # equal SBUF base partitions (and at most one PSUM
                    # input), so the Re/Im squares go through ACT into two
                    # base-0 tiles and the DVE adds them.  Rows 48:64 are
                    # zero lhsT columns -> always initialized.
                    sq_re = sqp.tile([64, NFREE], f16)
                    sq_im = sqp.tile([64, NFREE], f16)
                    nc.scalar.square(sq_re, ps[0:64])
                    nc.scalar.square(sq_im, ps[64:128])
                    # Chunk pairs share one Sqrt/reduce pass (packed along
                    # the ss free dim); the drain chunks are unpaired so the
                    # final dependency chain is short.
                    if drain:
                        ss1 = ssp.tile([64, IPB, GJ], f16, name="ss1")
                        nc.vector.tensor_add(
                            ss1.rearrange("p i g -> p (i g)"), sq_re, sq_im
                        )
                        mag1 = magp.tile([64, IPB, GJ], f16)
                        nc.scalar.sqrt(mag1, ss1)
                        red1 = redp.tile([64, IPB], f32)
                        nc.vector.reduce_sum(
                            out=red1, in_=mag1, axis=mybir.AxisListType.X
                        )
                        ocol = out_acc[:, bt, c % 2]
                        nc.vector.tensor_add(ocol, ocol, red1)
                    else:
                        pair = c // 2
                        if pair not in ss_tiles:
                            ss_tiles[pair] = ssp.tile(
                                [64, 2, IPB, GJ], f16, name="ss"
                            )
                        ss = ss_tiles[pair]
                        nc.vector.tensor_add(
                            ss[:, c % 2].rearrange("p i g -> p (i g)"),
                            sq_re,
                            sq_im,
                        )
                        if c % 2 == 1:
                            mag = magp.tile([64, 2, IPB, GJ], f16)
                            nc.scalar.sqrt(mag, ss)
                            ocol = out_acc[:, bt]
                            if c % NCHUNK == 1:
                                nc.vector.reduce_sum(
                                    out=ocol, in_=mag, axis=mybir.AxisListType.X
                                )
                            else:
                                red = redp.tile([64, 2, IPB], f32)
                                nc.vector.reduce_sum(
                                    out=red, in_=mag, axis=mybir.AxisListType.X
                                )
                                nc.vector.tensor_add(ocol, ocol, red)
                    # Per-batch output DMA: batches 0/1 fly mid-stream; only
                    # batch 2's small DMA trails the final accumulation.
                    if c % NCHUNK == NCHUNK - 1:
                        nc.sync.dma_start(
                            out=out_d[:, bt], in_=out_acc[:, bt]
                        )

    nc.compile()
    return nc


def kernel(x: np.ndarray) -> np.ndarray:
    global _CACHED_NC, LAST_EXEC_NS, LAST_RESULTS
    x = np.ascontiguousarray(np.asarray(x, dtype=np.float32))
    assert x.shape == (B, C, H, W), x.shape

    if _CACHED_NC is None:
        _CACHED_NC = _build()
    nc = _CACHED_NC

    w = _weights()
    in_maps = [
        {"x": x[i * BL : (i + 1) * BL].reshape(NIMG, H, W), "w": w}
        for i in range(N_CORES)
    ]
    kwargs = dict(BENCH_KWARGS)
    if BENCH:
        kwargs.setdefault("trace", True)
    res = run_bass_kernel_spmd(nc, in_maps, core_ids=list(range(N_CORES)), **kwargs)
    LAST_EXEC_NS = res.exec_time_ns
    LAST_RESULTS = res

    outs = []
    for i in range(N_CORES):
        # [64, NBATCH, 2, IPB]: row = band*8+gi (rows 48:64 zero),
        # cols = (batch, chunk-parity, img-in-batch)
        o = np.asarray(res.results[i]["out"], dtype=np.float64)
        o = o[0:48].reshape(NBANDS, 8, NBATCH, 2, IPB)
        tot = o.sum(axis=(1, 3)).reshape(NBANDS, NIMG) / 4096.0  # [band, img]
        outs.append(tot.T.reshape(BL, C * NBANDS))  # img = b_l*C + ch
    return np.concatenate(outs, axis=0).astype(np.float32)
